# revision 1
# baseline (speedup 1.0000x reference)
"""Trainium2 Bass kernel for nn_ContrastiveDistortion (symmetric pairwise-KL InfoNCE loss).

Math: with IS_SYMMETRIC=True the logdet terms cancel and
  logits_sym[a,b] = D/2 - U[a,b]/4,  U = tr(a,b)+tr(b,a)+quad(a,b)+quad(b,a)
U is a single inner product of stacked feature planes (K=6*128) plus a rank-1
column term c_b (row term c_a cancels in log-softmax). Each of the 8 cores gets
the full [128,4096] feature-major mu/sigma, column-ROTATED by 512*k so that the
program is SPMD-identical: the core's own 512-row block is always local columns
0..511 (diagonal masked there) and the positive pairs are local columns
2048..2559. Row-block softmax uses two per-half online logsumexps combined at
the end; per-core partial row-loss sums are reduced on host.
"""

import sys
from contextlib import ExitStack

import numpy as np

sys.path.insert(0, "/opt/trn_rl_repo")

import concourse.bass as bass
import concourse.bacc as bacc_mod
import concourse.mybir as mybir
from concourse.bass_utils import run_bass_kernel_spmd
from concourse.tile import TileContext

F32 = mybir.dt.float32
F32R = mybir.dt.float32r
I32 = mybir.dt.int32
AF = mybir.ActivationFunctionType
ALU = mybir.AluOpType
AX = mybir.AxisListType

P = 128          # partitions / feature dim D
NB = 4096        # N = 2B rows
NC = 8           # cores
RB = NB // NC    # 512 rows per core
NM = RB // P     # 4 m-chunks of 128 rows
HALF = NB // 2   # 2048 columns per softmax half
TEMPERATURE = 0.1
WEIGHT = 5.0
SCL = 1.0 / (4.0 * TEMPERATURE)  # 2.5: l = -SCL*U + const_row


def _build_nc():
    nc = bacc_mod.Bacc(None, target_bir_lowering=False, name="contrastive_distortion")
    muT_d = nc.declare_dram_parameter("muT", [P, NB], F32R, isOutput=False)
    sgT_d = nc.declare_dram_parameter("sigmaT", [P, NB], F32R, isOutput=False)
    out_d = nc.declare_dram_parameter("out", [P, NM], F32, isOutput=True)

    with TileContext(nc) as tc, ExitStack() as ctx:
        big = ctx.enter_context(tc.tile_pool(name="big", bufs=1))
        sm = ctx.enter_context(tc.tile_pool(name="sm", bufs=1))
        scr = ctx.enter_context(tc.tile_pool(name="scr", bufs=2))
        pp = ctx.enter_context(tc.tile_pool(name="pp", bufs=2, space="PSUM"))

        # persistent planes, feature-major [128, 4096]
        mu = big.tile([P, NB], F32R)
        sg = big.tile([P, NB], F32R)
        var = big.tile([P, NB], F32R)
        inv = big.tile([P, NB], F32R)
        msq = big.tile([P, NB], F32R)
        m2i = big.tile([P, NB], F32R)
        miv = big.tile([P, NB], F32R)
        muv = big.tile([P, NB], F32R)
        oneh = big.tile([P, RB * NM], F32)  # [128, 2048] stripe one-hots per m

        ones = sm.tile([P, P], F32R)
        ones_f = sm.tile([P, P], F32)
        ioti = sm.tile([P, RB], I32)
        mrow8 = sm.tile([P, 8], F32)
        bias8 = sm.tile([P, 8], F32)
        esum8 = sm.tile([P, 8], F32)
        upos4 = sm.tile([P, NM], F32)

        nc.vector.memset(ones_f, 1.0)
        nc.vector.tensor_copy(out=ones, in_=ones_f)
        # ioti[p, c] = c - p ; onehot_m[p, c] = (c - p == 128*m)
        nc.gpsimd.iota(ioti, pattern=[[1, RB]], base=0, channel_multiplier=-1)
        for m in range(NM):
            nc.vector.tensor_single_scalar(
                out=oneh[:, RB * m:RB * (m + 1)], in_=ioti, scalar=P * m,
                op=ALU.is_equal)

        blk = slice(0, RB)  # this core's block columns (local cols 0..511)
        us_list = []
        c8 = 0
        for h in range(2):
            sl = slice(HALF * h, HALF * (h + 1))
            nc.sync.dma_start(out=mu[:, sl], in_=muT_d[:, sl])
            nc.sync.dma_start(out=sg[:, sl], in_=sgT_d[:, sl])
            nc.vector.tensor_mul(var[:, sl], sg[:, sl], sg[:, sl])
            with nc.allow_low_precision("planes feed the PE which reads fp22"):
                nc.vector.reciprocal(inv[:, sl], var[:, sl])
            nc.vector.tensor_mul(msq[:, sl], mu[:, sl], mu[:, sl])
            nc.vector.scalar_tensor_tensor(
                out=m2i[:, sl], in0=mu[:, sl], scalar=-2.0, in1=inv[:, sl],
                op0=ALU.mult, op1=ALU.mult)
            nc.vector.tensor_mul(miv[:, sl], msq[:, sl], inv[:, sl])
            nc.vector.tensor_copy(out=muv[:, sl], in_=mu[:, sl])

            # chunk 7 (ones, miv) adds c_b = sum_d mu^2*inv to every row of U
            chunks = [(inv, var), (inv, msq), (m2i, muv), (var, inv),
                      (msq, inv), (muv, m2i), (None, miv)]
            for m in range(NM):
                mblk = slice(P * m, P * (m + 1))
                u = pp.tile([P, HALF], F32, name=f"u{h}{m}", tag="ps")
                # Matmul instructions can carry only ONE sync wait in walrus
                # codegen. A psum-slot-reusing group head needs two deps:
                # WAW vs the old matmuls (PE sem) and WAR vs the old tile's
                # DVE read-out. This orphan bf16 ldweights (no PSUM write ->
                # no PE wait) absorbs the DVE dep via the dw token written
                # right after that read-out, leaving the real matmuls with
                # just the PE-completion wait.
                if c8 >= 2:
                    nc.tensor.ldweights(
                        us_list[c8 - 2].bitcast(mybir.dt.bfloat16)[0:1, 0:2])
                for jj in range(4):
                    osl = slice(RB * jj, RB * (jj + 1))
                    gsl = slice(HALF * h + RB * jj, HALF * h + RB * (jj + 1))
                    for ci, (lp, rp) in enumerate(chunks):
                        lhsT = ones if lp is None else lp[:, mblk]
                        nc.tensor.matmul(
                            u[:, osl], lhsT=lhsT, rhs=rp[:, gsl],
                            start=(ci == 0), stop=(ci == len(chunks) - 1))
                if h == 0:
                    # exclude the diagonal (always in local cols 0..511)
                    nc.vector.scalar_tensor_tensor(
                        out=u[:, 0:RB], in0=oneh[:, RB * m:RB * (m + 1)],
                        scalar=1e30, in1=u[:, 0:RB], op0=ALU.mult, op1=ALU.add)
                us = scr.tile([P, HALF], F32, name="us", tag="us", bufs=2)
                us_list.append(us)
                nc.vector.tensor_copy(out=us, in_=u)
                nc.vector.tensor_reduce(mrow8[:, c8:c8 + 1], us, axis=AX.X,
                                        op=ALU.min)
                if h == 1:
                    # positive logits live at local cols 2048+128m+p
                    s512 = scr.tile([P, RB], F32, name="s512", tag="s512",
                                    bufs=1)
                    nc.vector.tensor_mul(s512, us[:, 0:RB],
                                         oneh[:, RB * m:RB * (m + 1)])
                    nc.vector.tensor_reduce(upos4[:, m:m + 1], s512, axis=AX.X,
                                            op=ALU.add)
                nc.vector.tensor_scalar_mul(bias8[:, c8:c8 + 1],
                                            mrow8[:, c8:c8 + 1], SCL)
                e2k = scr.tile([P, HALF], F32, name="e2k", tag="e2k", bufs=1)
                nc.scalar.activation(
                    out=e2k, in_=us, func=AF.Exp, bias=bias8[:, c8:c8 + 1],
                    scale=-SCL, accum_out=esum8[:, c8:c8 + 1])
                c8 += 1

        # tail: per-row LSE = logaddexp(L0, L1); row_loss = LSE + SCL*upos
        logE8 = sm.tile([P, 8], F32)
        nc.scalar.activation(out=logE8, in_=esum8, func=AF.Ln)
        L8 = sm.tile([P, 8], F32)
        nc.vector.scalar_tensor_tensor(out=L8, in0=mrow8, scalar=-SCL,
                                       in1=logE8, op0=ALU.mult, op1=ALU.add)
        L8v = L8.rearrange("p (b a) -> p b a", b=2)  # [:, h, m]
        M4 = sm.tile([P, NM], F32)
        nc.vector.tensor_max(M4, L8v[:, 0, :], L8v[:, 1, :])
        dd = sm.tile([P, 2, NM], F32)
        nc.vector.tensor_sub(dd[:, 0, :], L8v[:, 0, :], M4)
        nc.vector.tensor_sub(dd[:, 1, :], L8v[:, 1, :], M4)
        ee = sm.tile([P, 2, NM], F32)
        nc.scalar.activation(out=ee, in_=dd, func=AF.Exp)
        S4 = sm.tile([P, NM], F32)
        nc.vector.tensor_add(S4, ee[:, 0, :], ee[:, 1, :])
        logS4 = sm.tile([P, NM], F32)
        nc.scalar.activation(out=logS4, in_=S4, func=AF.Ln)
        LSE4 = sm.tile([P, NM], F32)
        nc.vector.tensor_add(LSE4, M4, logS4)
        rl4 = sm.tile([P, NM], F32)
        nc.vector.scalar_tensor_tensor(out=rl4, in0=upos4, scalar=SCL,
                                       in1=LSE4, op0=ALU.mult, op1=ALU.add)
        nc.sync.dma_start(out=out_d[:, :], in_=rl4)

    return nc


_NC_CACHE = None


def _get_nc():
    global _NC_CACHE
    if _NC_CACHE is None:
        nc = _build_nc()
        nc.finalize()  # runs Bacc.compile(): wait legalization for TRN2
        _NC_CACHE = nc
    return _NC_CACHE


def run_sharded(mu_x, sigma_x, mu_p, sigma_p, trace=False):
    mus = np.concatenate([np.asarray(mu_x, np.float32),
                          np.asarray(mu_p, np.float32)], 0)
    sigmas = np.concatenate([np.asarray(sigma_x, np.float32),
                             np.asarray(sigma_p, np.float32)], 0)
    muT = np.ascontiguousarray(mus.T)
    sgT = np.ascontiguousarray(sigmas.T)
    in_maps = [
        {"muT": np.ascontiguousarray(np.roll(muT, -RB * k, axis=1)),
         "sigmaT": np.ascontiguousarray(np.roll(sgT, -RB * k, axis=1))}
        for k in range(NC)
    ]
    kwargs = {}
    if trace:
        kwargs = dict(trace=True, trace_cores=[0])
    br = run_bass_kernel_spmd(_get_nc(), in_maps, core_ids=list(range(NC)),
                              **kwargs)
    total = sum(float(r["out"].astype(np.float64).sum()) for r in br.results)
    n_classes = NB - 1
    to_mult = (n_classes - 1.0 / WEIGHT) / (n_classes - 1)
    to_add = -np.log(np.float32(to_mult))
    loss = np.float32(total / NB - to_add)
    return loss, br


def kernel(z_hat, mu_x, sigma_x, mu_p, sigma_p):
    loss, _ = run_sharded(mu_x, sigma_x, mu_p, sigma_p)
    return np.asarray(loss, np.float32)



# revision 10
# speedup vs baseline: 1.1707x; 1.1707x over previous
"""Trainium2 Bass kernel for nn_ContrastiveDistortion (symmetric pairwise-KL InfoNCE loss).

Math: with IS_SYMMETRIC=True the logdet terms cancel and
  logits_sym[a,b] = D/2 - U[a,b]/4,
  U[a,b] = <inv_a, q_b> + <q_a, inv_b> + <m2i_a, mu_b> + <mu_a, m2i_b> + c_a + c_b
with q = var + mu^2, m2i = -2*mu*inv, c = sum_d mu^2*inv. c_a is a row
constant (cancels in log-softmax); c_b is computed on the HOST and shipped as a
partition-broadcast plane, so the PE only runs 4 K=128 chunks per output tile
(the baseline ran 7). Each of the 8 cores gets the full [128,4096]
feature-major mu/sigma, column-ROTATED by 512*k so the program is
SPMD-identical: the core's own 512-row block is local columns 0..511 (diagonal
masked there) and the positive pairs are local columns 2048..2559.

Per 512-row block the NxN work streams through PSUM in [128,1024] slabs
(4 slabs in flight = all 8 banks). A single fused DVE tensor_tensor_reduce per
slab drains PSUM: us = (u + c_b)*SCL with an online min accumulated across the
two slabs of each half-row tile. ACT exponentiates with bias=min and row-sum
accum; Pool handles one-hot positive extraction and part of the preprocessing.
Per-row partials (min, expsum, pos-logit) are shipped to the host, which does
the final logsumexp/mean in float64.
"""

import sys
from contextlib import ExitStack

import numpy as np

sys.path.insert(0, "/opt/trn_rl_repo")

import concourse.bass as bass
import concourse.bacc as bacc_mod
import concourse.mybir as mybir
from concourse.bass_utils import run_bass_kernel_spmd
from concourse.tile import TileContext

F32 = mybir.dt.float32
F32R = mybir.dt.float32r
I32 = mybir.dt.int32
BF16 = mybir.dt.bfloat16
AF = mybir.ActivationFunctionType
ALU = mybir.AluOpType
AX = mybir.AxisListType

P = 128          # partitions / feature dim D
NB = 4096        # N = 2B rows
NC = 8           # cores
RB = NB // NC    # 512 rows per core
NM = RB // P     # 4 m-chunks of 128 rows
HALF = NB // 2   # 2048 columns per softmax half
SLAB = 1024      # PSUM slab width (2 banks)
TEMPERATURE = 0.1
WEIGHT = 5.0
SCL = 1.0 / (4.0 * TEMPERATURE)  # 2.5: l = -SCL*U + const_row


def _build_nc():
    nc = bacc_mod.Bacc(None, target_bir_lowering=False, name="contrastive_distortion")
    muT_d = nc.declare_dram_parameter("muT", [P, NB], F32R, isOutput=False)
    sgT_d = nc.declare_dram_parameter("sigmaT", [P, NB], F32R, isOutput=False)
    cbB_d = nc.declare_dram_parameter("cbB", [P, NB], F32, isOutput=False)
    out_d = nc.declare_dram_parameter("out", [P, 20], F32, isOutput=True)

    with TileContext(nc) as tc, ExitStack() as ctx:
        big = ctx.enter_context(tc.tile_pool(name="big", bufs=1))
        sm = ctx.enter_context(tc.tile_pool(name="sm", bufs=1))
        scr = ctx.enter_context(tc.tile_pool(name="scr", bufs=2))
        pp = ctx.enter_context(tc.tile_pool(name="pp", bufs=4, space="PSUM"))

        # persistent planes, feature-major [128, 4096]
        mu = big.tile([P, NB], F32R)
        sg = big.tile([P, NB], F32R)
        var = big.tile([P, NB], F32R)
        msq = big.tile([P, NB], F32R)
        inv = big.tile([P, NB], F32R)
        q = big.tile([P, NB], F32R)
        mi = big.tile([P, NB], F32R)
        mud = big.tile([P, NB], F32R)
        cb = big.tile([P, NB], F32)
        oneh = big.tile([P, RB * NM], F32)  # [128, 2048] stripe one-hots per m

        ioti = sm.tile([P, RB], I32)
        mn_tmp = sm.tile([P, 8], F32)
        mn2 = sm.tile([P, 8], F32)
        outpack = sm.tile([P, 20], F32)  # 0:8 min, 8:16 expsum, 16:20 upos
        e2k = sm.tile([P, HALF], F32)
        junk = sm.tile([P, RB], F32)

        # ioti[p, c] = c - p ; onehot_m[p, c] = (c - p == 128*m)
        nc.gpsimd.iota(ioti, pattern=[[1, RB]], base=0, channel_multiplier=-1)
        for m in range(NM):
            nc.vector.tensor_single_scalar(
                out=oneh[:, RB * m:RB * (m + 1)], in_=ioti, scalar=P * m,
                op=ALU.is_equal)

        # input DMAs in 1024-col quarters; cb on the ACT queue so it streams
        # in parallel with mu/sg on the sync queue
        for qd in range(4):
            sl = slice(SLAB * qd, SLAB * (qd + 1))
            nc.sync.dma_start(out=mu[:, sl], in_=muT_d[:, sl])
            nc.sync.dma_start(out=sg[:, sl], in_=sgT_d[:, sl])
            nc.sync.dma_start(out=cb[:, sl], in_=cbB_d[:, sl])

        # preprocessing, 1024-col chunks, spread across ACT/DVE/Pool
        # (Pool only supports plain tensor_tensor; the -2 of the classic
        # m2i plane is folded into mud = -2*mu so U = <inv,q>+<q,inv>
        # +<mi,mud>+<mud,mi> + c_a + c_b with mi = mu*inv)
        for qd in range(4):
            sl = slice(SLAB * qd, SLAB * (qd + 1))
            nc.scalar.activation(out=var[:, sl], in_=sg[:, sl],
                                 func=AF.Square)
            nc.scalar.activation(out=mud[:, sl], in_=mu[:, sl],
                                 func=AF.Copy, scale=-2.0)
            nc.vector.tensor_mul(msq[:, sl], mu[:, sl], mu[:, sl])
            with nc.allow_low_precision("planes feed the PE which reads fp22"):
                nc.vector.reciprocal(inv[:, sl], var[:, sl])
            nc.vector.tensor_add(q[:, sl], var[:, sl], msq[:, sl])
            nc.vector.tensor_mul(mi[:, sl], mu[:, sl], inv[:, sl])

        chunks = [(inv, q), (q, inv), (mi, mud), (mud, mi)]
        slab_release = []  # SBUF AP whose write releases that psum slot
        si = 0
        for h in range(2):
            for m in range(NM):
                hm = 4 * h + m
                us = scr.tile([P, HALF], F32, name=f"us{hm}", tag="us", bufs=2)
                for s in range(2):
                    u = pp.tile([P, SLAB], F32, name=f"u{h}{m}{s}", tag="ps")
                    # Matmul instructions can carry only ONE sync wait in
                    # walrus codegen. A psum-slot-reusing group head needs two
                    # deps: WAW vs the old matmuls (PE sem) and WAR vs the old
                    # slab's DVE read-out. This orphan bf16 ldweights (no PSUM
                    # write -> no PE wait) absorbs the DVE dep via the token
                    # written right after that read-out, leaving the real
                    # matmuls with just the PE-completion wait.
                    if si >= 4:
                        nc.tensor.ldweights(slab_release[si - 4])
                    for j in range(2):
                        osl = slice(512 * j, 512 * (j + 1))
                        gsl = slice(HALF * h + SLAB * s + 512 * j,
                                    HALF * h + SLAB * s + 512 * (j + 1))
                        for ci, (lp, rp) in enumerate(chunks):
                            nc.tensor.matmul(
                                u[:, osl], lhsT=lp[:, P * m:P * (m + 1)],
                                rhs=rp[:, gsl],
                                start=(ci == 0), stop=(ci == len(chunks) - 1))
                    if h == 0 and s == 0:
                        # exclude the diagonal (always in local cols 0..511)
                        nc.vector.scalar_tensor_tensor(
                            out=u[:, 0:RB], in0=oneh[:, RB * m:RB * (m + 1)],
                            scalar=1e30, in1=u[:, 0:RB],
                            op0=ALU.mult, op1=ALU.add)
                    ssl = slice(SLAB * s, SLAB * (s + 1))
                    csl = slice(HALF * h + SLAB * s, HALF * h + SLAB * (s + 1))
                    acc = mn_tmp[:, hm:hm + 1] if s == 0 else mn2[:, hm:hm + 1]
                    nc.vector.scalar_tensor_tensor(
                        out=us[:, ssl], in0=u, scalar=SCL, in1=cb[:, csl],
                        op0=ALU.mult, op1=ALU.add)
                    nc.vector.tensor_reduce(acc, us[:, ssl], axis=AX.X,
                                            op=ALU.min)
                    slab_release.append(us.bitcast(BF16)[0:1, 2 * SLAB * s:
                                                         2 * SLAB * s + 2])
                    si += 1
                nc.vector.tensor_tensor(
                    out=outpack[:, hm:hm + 1], in0=mn_tmp[:, hm:hm + 1],
                    in1=mn2[:, hm:hm + 1], op=ALU.min)
                # exp(-(us - min)) with row-sum accumulated into outpack
                nc.scalar.activation(
                    out=e2k, in_=us, func=AF.Exp, bias=outpack[:, hm:hm + 1],
                    scale=-1.0, accum_out=outpack[:, 8 + hm:9 + hm])
                if h == 1:
                    # positive logits live at local cols 2048+128m+p
                    s512 = scr.tile([P, RB], F32, name=f"s512{m}", tag="s512",
                                    bufs=2)
                    nc.vector.tensor_mul(s512, us[:, 0:RB],
                                         oneh[:, RB * m:RB * (m + 1)])
                    nc.scalar.activation(
                        out=junk, in_=s512, func=AF.Identity,
                        accum_out=outpack[:, 16 + m:17 + m])
        nc.sync.dma_start(out=out_d[:, :], in_=outpack)

    return nc


_NC_CACHE = None


def _get_nc():
    global _NC_CACHE
    if _NC_CACHE is None:
        nc = _build_nc()
        nc.finalize()  # runs Bacc.compile(): wait legalization for TRN2
        _NC_CACHE = nc
    return _NC_CACHE


def run_sharded(mu_x, sigma_x, mu_p, sigma_p, trace=False):
    mus = np.concatenate([np.asarray(mu_x, np.float32),
                          np.asarray(mu_p, np.float32)], 0)
    sigmas = np.concatenate([np.asarray(sigma_x, np.float32),
                             np.asarray(sigma_p, np.float32)], 0)
    muT = np.ascontiguousarray(mus.T)
    sgT = np.ascontiguousarray(sigmas.T)
    # c_b = sum_d mu^2 / var, exact in f64 on the host
    c = ((mus.astype(np.float64) ** 2)
         / (sigmas.astype(np.float64) ** 2)).sum(1)
    c = (c * SCL).astype(np.float32)
    in_maps = []
    for k in range(NC):
        ck = np.roll(c, -RB * k)
        in_maps.append({
            "muT": np.ascontiguousarray(np.roll(muT, -RB * k, axis=1)),
            "sigmaT": np.ascontiguousarray(np.roll(sgT, -RB * k, axis=1)),
            "cbB": np.ascontiguousarray(
                np.broadcast_to(ck[None, :], (P, NB)).astype(np.float32)),
        })
    kwargs = {}
    if trace:
        kwargs = dict(trace=True, trace_cores=[0])
    br = run_bass_kernel_spmd(_get_nc(), in_maps, core_ids=list(range(NC)),
                              **kwargs)
    outs = np.stack([np.asarray(r["out"], np.float64) for r in br.results])
    mn = outs[:, :, 0:8]     # [8,128,8]: per-(h,m) min of us = SCL*(U+c_b)
    es = outs[:, :, 8:16]    # per-(h,m) sum exp(min - us)
    up = outs[:, :, 16:20]   # [8,128,4]: us at the positive column
    L = -mn + np.log(es)     # per-half log sum exp(-us)
    LSE = np.logaddexp(L[:, :, 0:4], L[:, :, 4:8])
    rl = LSE + up            # row loss (row consts cancel)
    n_classes = NB - 1
    to_mult = (n_classes - 1.0 / WEIGHT) / (n_classes - 1)
    to_add = -np.log(np.float32(to_mult))
    loss = np.float32(rl.sum() / NB - to_add)
    return loss, br


def kernel(z_hat, mu_x, sigma_x, mu_p, sigma_p):
    loss, _ = run_sharded(mu_x, sigma_x, mu_p, sigma_p)
    return np.asarray(loss, np.float32)
